# revision 15
# baseline (speedup 1.0000x reference)
"""AdaConv (topk_masking) Trainium2 kernel, 8-core data parallel.

Sharding: 8 cores = 4 batches x 2 H-halves; each core computes a 64-row half
image with a 2-row reflect halo. Per pixel: top-9-of-25 smallest distances in
the 5x5 window, gather the selected x columns, contract against rank-ordered
[O, C*9] weights + bias.

Per-core pipeline:
  keys = -(round(l*2^19')*32 + tag) on a transposed padded grid, where
  tag = (y%5)*5 + (x%5) uniquely labels cells within any 5x5 window;
  DVE max/match_replace extracts the 9 smallest keys per pixel in rank
  order; arithmetic decode turns keys back into flat gather indices;
  x is cast to bf16 and PE-transposed into [spatial, 128B] rows staged in
  DRAM; gpsimd dma_gather (transpose mode) lands G[c, (y,t,x)] in SBUF;
  PE contracts (c+bias-lane, t) with PSUM accumulation; ACT evacuates.
"""
import sys
import os
import numpy as np

sys.path.insert(0, '/opt/trn_rl_repo')
KSTAGE = int(os.environ.get('KSTAGE', '9'))

B, C, O, H, W = 4, 64, 64, 128, 128
KS, WS = 3, 5
K2 = KS * KS
P = (WS - 1) // 2
HH = H // 2              # 64 output rows per core
PH = HH + 2 * P          # 68 padded rows
PW = W + 2 * P           # 132 padded cols
M = PH * PW              # 8976
NBLK = (M + 127) // 128  # 71
MPAD = NBLK * 128
SCALE = float(2 ** 19 - 64)
MAGIC = 8388608.0
NIDX = HH * K2 * W       # 73728
CHUNKS = 8
YPC = HH // CHUNKS       # 8
NI_C = YPC * K2 * W      # 9216
KCHUNKS = int(os.environ.get('KCHUNKS', str(CHUNKS)))

_CACHE = {}


def _win_ap(tens, p0, npart, free_len):
    """Overlapping AP view [npart, HH, 5]: [p, y, dy] = tens[p0+p, y+dy]."""
    import bass_rust
    ap = tens[p0:p0 + npart, :].copy()
    pairs = list(ap.ap)
    ap.ap = bass_rust.VecI64Pair([list(pairs[0][:1]) + [npart] if False else [pairs[0][0], npart],
                                  [1, HH], [1, 5]])
    return ap


def _build_program():
    from concourse import bacc, mybir, library_config

    F32 = mybir.dt.float32
    BF16 = mybir.dt.bfloat16
    I16 = mybir.dt.int16
    ALU = mybir.AluOpType
    ACTF = mybir.ActivationFunctionType

    nc = bacc.Bacc()

    xs_in = nc.declare_dram_parameter("xs", [C, M], F32, isOutput=False)
    lpa_in = nc.declare_dram_parameter("lpa", [128, PH], F32, isOutput=False)
    lpb_in = nc.declare_dram_parameter("lpb", [4, PH], F32, isOutput=False)
    taga_in = nc.declare_dram_parameter("taga", [128, PH], F32, isOutput=False)
    tagb_in = nc.declare_dram_parameter("tagb", [4, PH], F32, isOutput=False)
    ymod_in = nc.declare_dram_parameter("ymod", [128, HH * K2], F32, isOutput=False)
    base_in = nc.declare_dram_parameter("base", [128, HH * K2], F32, isOutput=False)
    xmod_in = nc.declare_dram_parameter("xmod", [128, 1], F32, isOutput=False)
    w_in = nc.declare_dram_parameter("wcat", [C + 1, K2 * O], F32, isOutput=False)
    id_in = nc.declare_dram_parameter("ident", [C, C], F32, isOutput=False)
    out_ext = nc.declare_dram_parameter("out", [O, HH * W], F32, isOutput=True)

    sem = nc.alloc_semaphore()
    sv = 0
    ALLENG = (nc.sync, nc.vector, nc.scalar, nc.gpsimd, nc.tensor)

    def allwait():
        for eng in ALLENG:
            eng.wait_ge(sem, sv)

    xs_sb = nc.alloc_sbuf_tensor("xs_sb", [C, M], F32)
    lpa = nc.alloc_sbuf_tensor("lpa_sb", [128, PH], F32)
    lpb = nc.alloc_sbuf_tensor("lpb_sb", [4, PH], F32)
    taga = nc.alloc_sbuf_tensor("taga_sb", [128, PH], F32)
    tagb = nc.alloc_sbuf_tensor("tagb_sb", [4, PH], F32)
    ymod = nc.alloc_sbuf_tensor("ymod_sb", [128, HH * K2], F32)
    base = nc.alloc_sbuf_tensor("base_sb", [128, HH * K2], F32)
    xmod = nc.alloc_sbuf_tensor("xmod_sb", [128, 1], F32)
    w_sb = nc.alloc_sbuf_tensor("w_sb", [C + 1, K2 * O], F32)
    w_bf = nc.alloc_sbuf_tensor("w_bf", [C + 1, K2 * O], BF16)
    id_sb = nc.alloc_sbuf_tensor("id_sb", [C, C], F32)
    id_bf = nc.alloc_sbuf_tensor("id_bf", [C, C], BF16)

    for dst, src in ((xs_sb, xs_in), (lpa, lpa_in), (lpb, lpb_in), (taga, taga_in),
                     (tagb, tagb_in), (ymod, ymod_in), (base, base_in),
                     (xmod, xmod_in), (w_sb, w_in), (id_sb, id_in)):
        nc.sync.dma_start(dst[:], src[:]).then_inc(sem, 16)
        sv += 16
    allwait()

    V = nc.vector
    STT = nc.vector.scalar_tensor_tensor

    # ---- keys on transposed [x, y] grid ----
    ka = nc.alloc_sbuf_tensor("ka", [128, PH], F32)
    kb = nc.alloc_sbuf_tensor("kb", [4, PH], F32)
    tmpa = nc.alloc_sbuf_tensor("tmpa", [128, PH], F32)
    tmpb = nc.alloc_sbuf_tensor("tmpb", [4, PH], F32)
    V.tensor_scalar(out=tmpa[:], in0=lpa[:], scalar1=SCALE, scalar2=MAGIC, op0=ALU.mult, op1=ALU.add)
    V.tensor_scalar(out=tmpb[:], in0=lpb[:], scalar1=SCALE, scalar2=MAGIC, op0=ALU.mult, op1=ALU.add)
    V.drain()
    V.tensor_scalar(out=tmpa[:], in0=tmpa[:], scalar1=MAGIC, scalar2=None, op0=ALU.subtract)
    V.tensor_scalar(out=tmpb[:], in0=tmpb[:], scalar1=MAGIC, scalar2=None, op0=ALU.subtract)
    V.drain()
    STT(out=ka[:], in0=tmpa[:], scalar=-32.0, in1=taga[:], op0=ALU.mult, op1=ALU.subtract)
    STT(out=kb[:], in0=tmpb[:], scalar=-32.0, in1=tagb[:], op0=ALU.mult, op1=ALU.subtract)
    V.drain()
    # concurrent ACT casts
    nc.scalar.activation(out=w_bf[:], in_=w_sb[:], func=ACTF.Copy)
    nc.scalar.activation(out=id_bf[:], in_=id_sb[:], func=ACTF.Copy)
    x_bf = nc.alloc_sbuf_tensor("x_bf", [C, M], BF16)
    nc.scalar.activation(out=x_bf[:], in_=xs_sb[:], func=ACTF.Copy).then_inc(sem, 1)
    sv += 1
    V.engine_nop().then_inc(sem, 1)
    sv += 1
    allwait()

    # ---- 25-wide window slab KW[x, y*25 + dx*5+dy] ----
    KW = nc.alloc_sbuf_tensor("KW", [128, HH * 25], F32)
    kwv = KW[:].rearrange("p (y j) -> p y j", y=HH)
    for dx in range(5):
        npart = 128 - dx
        nc.sync.dma_start(kwv[0:npart, :, dx * 5:dx * 5 + 5],
                          _win_ap(ka, dx, npart, PH)).then_inc(sem, 16)
        sv += 16
        if dx > 0:
            nc.sync.dma_start(kwv[npart:128, :, dx * 5:dx * 5 + 5],
                              _win_ap(kb, 0, dx, PH)).then_inc(sem, 16)
            sv += 16
    allwait()

    # ---- per-row top-9 (3 passes with drains) ----
    TH = nc.alloc_sbuf_tensor("TH", [128, HH * 16], F32)
    KWS = nc.alloc_sbuf_tensor("KWS", [128, HH * 25], F32)
    V.drain()
    for y in range(HH):
        V.max(out=TH[:, y * 16:y * 16 + 8], in_=KW[:, y * 25:(y + 1) * 25])
    V.drain()
    for y in range(HH):
        V.match_replace(out=KWS[:, y * 25:(y + 1) * 25], in_to_replace=TH[:, y * 16:y * 16 + 8],
                        in_values=KW[:, y * 25:(y + 1) * 25], imm_value=-1e30)
    V.drain()
    for y in range(HH):
        V.max(out=TH[:, y * 16 + 8:y * 16 + 16], in_=KWS[:, y * 25:(y + 1) * 25])
    V.drain()

    # ---- decode keys -> flat indices ----
    NYT = HH * K2
    th9 = TH[:].rearrange("p (y s) -> p y s", y=HH)[:, :, 0:K2]

    def v3(t):
        return t[:].rearrange("p (y s) -> p y s", y=HH)

    ql = nc.alloc_sbuf_tensor("ql", [128, NYT], F32)
    tg = nc.alloc_sbuf_tensor("tg", [128, NYT], F32)
    ty = nc.alloc_sbuf_tensor("ty", [128, NYT], F32)
    tx = nc.alloc_sbuf_tensor("tx", [128, NYT], F32)
    u1 = nc.alloc_sbuf_tensor("u1", [128, NYT], F32)
    s1 = nc.alloc_sbuf_tensor("s1", [128, NYT], F32)
    dyb = nc.alloc_sbuf_tensor("dyb", [128, NYT], F32)
    dxb = nc.alloc_sbuf_tensor("dxb", [128, NYT], F32)
    idxf = nc.alloc_sbuf_tensor("idxf", [128, NYT], F32)
    idx16 = nc.alloc_sbuf_tensor("idx16", [128, NYT], I16)
    u2 = nc.alloc_sbuf_tensor("u2", [128, NYT], F32)
    s2 = nc.alloc_sbuf_tensor("s2", [128, NYT], F32)
    V.tensor_scalar(out=v3(ql), in0=th9, scalar1=-1.0 / 32.0, scalar2=0.625, op0=ALU.mult, op1=ALU.add)
    V.drain()
    V.tensor_scalar(out=ql[:], in0=ql[:], scalar1=MAGIC, scalar2=MAGIC + 1.0, op0=ALU.add, op1=ALU.subtract)
    V.drain()
    STT(out=v3(tg), in0=v3(ql), scalar=-32.0, in1=th9, op0=ALU.mult, op1=ALU.subtract)
    V.drain()
    V.tensor_scalar(out=ty[:], in0=tg[:], scalar1=0.2, scalar2=0.6, op0=ALU.mult, op1=ALU.add)
    V.drain()
    V.tensor_scalar(out=ty[:], in0=ty[:], scalar1=MAGIC, scalar2=MAGIC + 1.0, op0=ALU.add, op1=ALU.subtract)
    V.drain()
    STT(out=tx[:], in0=ty[:], scalar=-5.0, in1=tg[:], op0=ALU.mult, op1=ALU.add)
    nc.vector.tensor_sub(u1[:], ty[:], ymod[:])
    V.drain()
    V.tensor_scalar(out=s1[:], in0=u1[:], scalar1=0.0, scalar2=None, op0=ALU.is_lt)
    V.tensor_scalar(out=u2[:], in0=tx[:], scalar1=xmod[:, 0:1], scalar2=None, op0=ALU.subtract)
    V.drain()
    STT(out=dyb[:], in0=s1[:], scalar=5.0, in1=u1[:], op0=ALU.mult, op1=ALU.add)
    V.tensor_scalar(out=s2[:], in0=u2[:], scalar1=0.0, scalar2=None, op0=ALU.is_lt)
    V.drain()
    STT(out=dxb[:], in0=s2[:], scalar=5.0, in1=u2[:], op0=ALU.mult, op1=ALU.add)
    V.drain()
    STT(out=idxf[:], in0=dyb[:], scalar=float(PW), in1=dxb[:], op0=ALU.mult, op1=ALU.add)
    V.drain()
    nc.vector.tensor_add(idxf[:], idxf[:], base[:])
    V.drain()
    V.tensor_scalar(out=idxf[:], in0=idxf[:], scalar1=0.0, scalar2=float(M - 1), op0=ALU.max, op1=ALU.min)
    V.drain()
    nc.vector.tensor_copy(idx16[:], idxf[:])
    V.engine_nop().then_inc(sem, 1)
    sv += 1
    allwait()

    # ---- wrap idx layout: i=(y*9+t)*128+x -> [i%16 (x8 groups), i//16] ----
    wrap = nc.alloc_sbuf_tensor("wrap", [128, NIDX // 16], I16)
    wrv = wrap[:].rearrange("p (yt e) -> p yt e", yt=NYT)
    with nc.allow_non_contiguous_dma(reason="idx wrap scatter"):
        for g in range(8):
            nc.sync.dma_start(
                wrv[0:16, :, g:g + 1],
                idx16[16 * g:16 * g + 16, :].rearrange("p (yt one) -> p yt one", one=1),
            ).then_inc(sem, 16)
            sv += 16
    allwait()
    for g in range(1, 8):
        nc.sync.dma_start(wrap[16 * g:16 * g + 16, :], wrap[0:16, :]).then_inc(sem, 16)
        sv += 16
    allwait()

    if KSTAGE < 2:
        nc.sync.dma_start(out_ext[:, 0:NYT], idxf[:, 0:NYT][0:O, :] if O <= 128 else None).then_inc(sem, 16)
        sv += 16
        allwait()
        nc.finalize()
        return nc

    # ---- xT via PE transpose, staged to DRAM ----
    xT = nc.alloc_sbuf_tensor("xT", [128, NBLK * 128], BF16)
    xtv = xT[:].rearrange("p (b e) -> p b e", b=NBLK)
    V.memset(xT[0:64, :], 0.0)
    V.memset(xT[64:128, :], 0.0)
    V.engine_nop().then_inc(sem, 1)
    sv += 1
    allwait()
    V.memset(xtv[:, :, C:C + 1], 1.0)
    V.engine_nop().then_inc(sem, 1)
    sv += 1
    allwait()
    tsem = nc.alloc_semaphore()
    ssem = nc.alloc_semaphore()
    pst0 = nc.alloc_psum_tensor("pst0", [128, C], BF16)
    pst1 = nc.alloc_psum_tensor("pst1", [128, C], BF16)
    psts = (pst0, pst1)
    for b in range(NBLK):
        wdt = 128 if b < NBLK - 1 else (M - 128 * (NBLK - 1))
        ps_b = psts[b % 2]
        if b >= 2:
            nc.tensor.wait_ge(ssem, b - 1)
        nc.tensor.transpose(out=ps_b[0:wdt, :], in_=x_bf[:, 128 * b:128 * b + wdt],
                            identity=id_bf[:]).then_inc(tsem, 1)
        nc.scalar.wait_ge(tsem, b + 1)
        nc.scalar.activation(out=xtv[0:wdt, b, 0:C], in_=ps_b[0:wdt, :],
                             func=ACTF.Copy).then_inc(ssem, 1)
    nc.scalar.wait_ge(ssem, NBLK)
    nc.scalar.activation(out=tmpb[:], in_=lpb[:], func=ACTF.Copy).then_inc(sem, 1)
    sv += 1
    allwait()
    xT_dram = nc.dram_tensor("xT_dram", [MPAD, 128], BF16)
    nc.sync.dma_start(xT_dram[:].rearrange("(b p) e -> p b e", p=128), xtv[:]).then_inc(sem, 16)
    sv += 16
    allwait()

    if KSTAGE < 3:
        nc.sync.dma_start(out_ext[:, 0:128], xT[0:O, 0:256].bitcast(F32)).then_inc(sem, 16)
        sv += 16
        allwait()
        nc.finalize()
        return nc

    # ---- chunked gather + conv ----
    nc.gpsimd.load_library(library_config.mlp)
    G0 = nc.alloc_sbuf_tensor("G0", [128, NI_C], BF16)
    G1 = nc.alloc_sbuf_tensor("G1", [128, NI_C], BF16)
    Gs = (G0, G1)
    gsem = nc.alloc_semaphore()
    csem = nc.alloc_semaphore()
    osem = nc.alloc_semaphore()
    out_sb = nc.alloc_sbuf_tensor("out_sb", [O, HH * W], F32)
    orow = out_sb[:].rearrange("o (y x) -> o y x", y=HH)
    ps_a = nc.alloc_psum_tensor("ps_a", [O, 512], F32)
    ps_c = nc.alloc_psum_tensor("ps_c", [O, 512], F32)
    for c in range(KCHUNKS):
        g_sb = Gs[c % 2]
        SUB = 768
        NSUB = NI_C // SUB  # 12
        if c >= 1:
            nc.gpsimd.wait_ge(gsem, 16 * NSUB * c)
        if c >= 2:
            nc.gpsimd.wait_ge(csem, c - 1)
        gvs = g_sb[:].rearrange("p (s n) -> p s n", s=NSUB)
        for s_ in range(NSUB):
            nc.gpsimd.dma_gather(
                out_ap=gvs[:, s_:s_ + 1, :],
                in_ap=xT_dram[:],
                idxs_ap=wrap[:, (NI_C // 16) * c + (SUB // 16) * s_:
                             (NI_C // 16) * c + (SUB // 16) * (s_ + 1)],
                num_idxs=SUB,
                num_idxs_reg=SUB,
                elem_size=128,
                transpose=True,
            ).then_inc(gsem, 16)
        nc.tensor.wait_ge(gsem, 16 * NSUB * (c + 1))
        if c >= 1:
            nc.tensor.wait_ge(osem, c)
        gv = g_sb[:].rearrange("p (y t x) -> p y t x", y=YPC, t=K2)
        for half, psu in ((0, ps_a), (1, ps_c)):
            for t in range(K2):
                if KSTAGE >= 4:
                    rhs_ap = gv[0:C + 1, YPC // 2 * half:YPC // 2 * (half + 1), t, :]
                else:
                    rhs_ap = xT[0:C + 1, 0:512]
                mm = nc.tensor.matmul(
                    out=psu[:],
                    lhsT=w_bf[:, O * t:O * (t + 1)],
                    rhs=rhs_ap,
                    start=(t == 0), stop=(t == K2 - 1),
                )
        mm.then_inc(csem, 1)
        nc.scalar.wait_ge(csem, c + 1)
        y0 = YPC * c
        nc.scalar.activation(out=orow[:, y0:y0 + YPC // 2, :],
                             in_=ps_a[:].rearrange("o (y x) -> o y x", y=YPC // 2),
                             func=ACTF.Copy)
        nc.scalar.activation(out=orow[:, y0 + YPC // 2:y0 + YPC, :],
                             in_=ps_c[:].rearrange("o (y x) -> o y x", y=YPC // 2),
                             func=ACTF.Copy).then_inc(osem, 1)
        nc.sync.wait_ge(osem, c + 1)
        nc.sync.dma_start(
            out_ext[:].rearrange("o (y x) -> o y x", y=HH)[:, y0:y0 + YPC, :],
            orow[:, y0:y0 + YPC, :],
        ).then_inc(sem, 16)
        sv += 16
    allwait()
    nc.finalize()
    return nc


def _host_prep(x, l, weight, bias):
    xp = np.pad(x, ((0, 0), (0, 0), (P, P), (P, P)), mode='reflect')
    lp = np.pad(l[:, 0], ((0, 0), (P, P), (P, P)), mode='constant', constant_values=999.0)
    yy = np.arange(PH)
    xx = np.arange(PW)
    tagT = ((yy[None, :] % 5) * 5 + (xx[:, None] % 5)).astype(np.float32)  # [PW, PH]
    ys = np.arange(HH)
    ymod = np.broadcast_to(np.repeat((ys % 5).astype(np.float32), K2)[None, :],
                           (128, HH * K2)).copy()
    basep = (np.repeat((ys * PW).astype(np.float32), K2)[None, :]
             + np.arange(128, dtype=np.float32)[:, None])
    basep = np.ascontiguousarray(basep, dtype=np.float32)
    xmod = (np.arange(128) % 5).astype(np.float32).reshape(128, 1)
    w2 = weight.reshape(O, C, K2).astype(np.float32)
    wcat = np.zeros((C + 1, K2 * O), dtype=np.float32)
    for t in range(K2):
        wcat[0:C, O * t:O * (t + 1)] = w2[:, :, t].T
    wcat[C, 0:O] = bias.astype(np.float32)
    ident = np.eye(C, dtype=np.float32)
    in_maps = []
    for core in range(8):
        b, half = core // 2, core % 2
        y0 = HH * half
        xs = xp[b, :, y0:y0 + PH, :].reshape(C, M).astype(np.float32)
        lpT = np.ascontiguousarray(lp[b, y0:y0 + PH, :].T, dtype=np.float32)  # [PW, PH]
        in_maps.append({
            "xs": np.ascontiguousarray(xs),
            "lpa": np.ascontiguousarray(lpT[0:128]),
            "lpb": np.ascontiguousarray(lpT[128:132]),
            "taga": np.ascontiguousarray(tagT[0:128]),
            "tagb": np.ascontiguousarray(tagT[128:132]),
            "ymod": ymod,
            "base": basep,
            "xmod": xmod,
            "wcat": wcat,
            "ident": ident,
        })
    return in_maps


def kernel(x, l, weight, bias, kernel_size, window_size, _profile=False):
    from concourse.bass_utils import run_bass_kernel_spmd
    assert int(kernel_size) == KS and int(window_size) == WS
    x = np.asarray(x, dtype=np.float32)
    l = np.asarray(l, dtype=np.float32)
    weight = np.asarray(weight, dtype=np.float32)
    bias = np.asarray(bias, dtype=np.float32)
    if "nc" not in _CACHE:
        _CACHE["nc"] = _build_program()
    nc = _CACHE["nc"]
    in_maps = _host_prep(x, l, weight, bias)
    kw = dict(trace=True) if _profile else {}
    res = run_bass_kernel_spmd(nc, in_maps, core_ids=list(range(8)), **kw)
    out = np.zeros((B, O, H, W), dtype=np.float32)
    for core in range(8):
        b, half = core // 2, core % 2
        out[b, :, HH * half:HH * (half + 1), :] = res.results[core]["out"].reshape(O, HH, W)
    _CACHE["exec_time_ns"] = getattr(res, "exec_time_ns", None)
    return out
